# revision 53
# baseline (speedup 1.0000x reference)
"""Trainium2 Bass kernel for nn_Attention_39487929319718.

Module: x:(4,128,64,64) -> 1x1conv QKV (+16 memory tokens) -> 4-head
attention (d=32) over n=4112 tokens -> drop memory queries -> 1x1 conv out.

Sharding: 8 cores = 4 batches x 2 query-halves. Each core receives its
batch's full x (token-rolled so its own 2048 queries sit first), computes
K/V for all 4112 tokens, attention for its 2048 queries across all 4 heads,
and the full output projection for those tokens. No collectives.

Per-core layout highlights:
 - Scores are computed transposed (k-tokens on partitions) so softmax'd
   probs feed the AV matmul directly with no transposes.
 - exp() has no max-subtraction (|scores| < 3 for this data distribution),
   with the 1/sqrt(d) scale folded into the activation's free affine.
 - V^T carries an extra ones column so each AV accumulation also produces
   the softmax denominator row for free.
"""

import sys

sys.path.insert(0, "/opt/trn_rl_repo")

import numpy as np

B, C, H, W = 4, 128, 64, 64
HEADS, DH, MEM = 4, 32, 16
HW = H * W            # 4096
NT = HW + MEM         # 4112 tokens (keys/values)
QN = HW // 2          # 2048 queries per core
SCALE = float(DH) ** -0.5

# k-token chunks of 128 (last chunk = 16 memory tokens)
KCHUNKS = [(j * 128, 128) for j in range(32)] + [(4096, 16)]
VTW = 33  # per-head vT columns per chunk: 32 v dims + 1 ones
VCW = HEADS * VTW  # 132 columns per chunk in the vT tile

_NC_CACHE = None


def _build_nc():
    import concourse.bacc as bacc
    import concourse.mybir as mybir
    import concourse.tile as tile

    F32 = mybir.dt.float32
    B16 = mybir.dt.bfloat16
    EXP = mybir.ActivationFunctionType.Exp

    nc = bacc.Bacc("TRN2", target_bir_lowering=False, debug=False, num_devices=8)

    xd = nc.dram_tensor("x", [C, HW], F32, kind="ExternalInput").ap()
    memd = nc.dram_tensor("mem", [C, MEM], F32, kind="ExternalInput").ap()
    wq_d = nc.dram_tensor("wqT", [C, 128], F32, kind="ExternalInput").ap()
    wk_d = nc.dram_tensor("wkT", [C, 128], F32, kind="ExternalInput").ap()
    wv_d = nc.dram_tensor("wvT", [C, 128], F32, kind="ExternalInput").ap()
    wo_d = nc.dram_tensor("woT", [128, C], F32, kind="ExternalInput").ap()
    bo_d = nc.dram_tensor("bout", [C, 1], F32, kind="ExternalInput").ap()
    outd = nc.dram_tensor("out", [C, QN], F32, kind="ExternalOutput").ap()

    with tile.TileContext(nc) as tc:
        with (
            tc.tile_pool(name="const", bufs=1) as constp,
            tc.tile_pool(name="big", bufs=1) as bigp,
            tc.tile_pool(name="pt", bufs=12) as ptp,
            tc.tile_pool(name="xin", bufs=3) as xinp,
            tc.tile_pool(name="ep", bufs=6) as epp,
            tc.tile_pool(name="simps", bufs=3, space="PSUM") as simp,
            tc.tile_pool(name="avps", bufs=2, space="PSUM") as avp,
        ):
            # ---- constants / weights (QKV weights cast to bf16) ----------
            # The first x chunk and the K/Q weights ride the sync DMA queue
            # (they gate the score stream); everything else takes the gpsimd
            # queue so the critical chain isn't serialized behind it.
            xe = bigp.tile([C, NT], B16, tag="xe")
            xf0 = xinp.tile([C, 512], F32, tag="xf", name="xf0")
            nc.sync.dma_start(out=xf0[:], in_=xd[:, 0:512])
            nc.vector.tensor_copy(xe[:, 0:512], xf0[:])

            wk_f = constp.tile([C, 128], F32, tag="wkf")
            nc.sync.dma_start(out=wk_f[:], in_=wk_d)
            wk_s = constp.tile([C, 128], B16, tag="wk")
            nc.vector.tensor_copy(wk_s[:], wk_f[:])
            wq_f = constp.tile([C, 128], F32, tag="wqf")
            nc.sync.dma_start(out=wq_f[:], in_=wq_d)
            wq_s = constp.tile([C, 128], B16, tag="wq")
            nc.vector.tensor_copy(wq_s[:], wq_f[:])
            wv_f = constp.tile([C, 128], F32, tag="wvf")
            nc.gpsimd.dma_start(out=wv_f[:], in_=wv_d)
            wv_s = constp.tile([C, 128], B16, tag="wv")
            nc.vector.tensor_copy(wv_s[:], wv_f[:])
            wo_f = constp.tile([128, C], F32, tag="wof")
            nc.gpsimd.dma_start(out=wo_f[:], in_=wo_d)
            wo_s = constp.tile([128, C], B16, tag="wo")
            nc.vector.tensor_copy(wo_s[:], wo_f[:])
            bo_s = constp.tile([C, 1], F32, tag="bo")
            nc.gpsimd.dma_start(out=bo_s[:], in_=bo_d)

            # ---- rest of x_ext, cast to bf16 chunk by chunk --------------
            for i in range(1, 8):
                xf = xinp.tile([C, 512], F32, tag="xf", name=f"xf{i}")
                eng = nc.sync if i % 2 == 0 else nc.gpsimd
                eng.dma_start(out=xf[:], in_=xd[:, i * 512 : (i + 1) * 512])
                nc.vector.tensor_copy(xe[:, i * 512 : (i + 1) * 512], xf[:])
            mf = xinp.tile([C, MEM], F32, tag="mf")
            nc.gpsimd.dma_start(out=mf[:], in_=memd)
            nc.vector.tensor_copy(xe[:, HW:NT], mf[:])

            # main-loop matmul operands in bf16 (accumulation stays fp32)
            Ksb = bigp.tile([C, NT], B16, tag="K")          # (4h x 32d, tok)
            Qsb = bigp.tile([C, QN], B16, tag="Q")          # (4h x 32d, q)
            VT = bigp.tile([C, 33, HEADS, VTW], B16, tag="VT")  # (tok%128, chunk, h, d+1)
            att = bigp.tile([C, QN], B16, tag="att")        # (4h x 32d, q) normalized
            osb = bigp.tile([C, QN], F32, tag="osb")

            # ---- projections ---------------------------------------------
            # Q: first 2048 columns (this core's queries);  K: all 4112 tokens
            # K/Q psum evacuation on ScalarE (idle during setup; DVE is busy
            # with the vT copies which gate the AV stream)
            # Projection psum comes from the sim pool; the avp pool is
            # reserved for AV accumulators so the AV stream starts at once.
            # K projections run through the avp pool (idle until the first
            # lagged AV fires) with psum evacuation on ScalarE; Q through the
            # sim pool on DVE. V-projection groups are emitted inside the
            # first query block's chunk loop so the score/exp stream starts
            # immediately and setup rides in PE slack.
            def qproj(i):
                ps = simp.tile([128, 1024], F32, tag="sim", name=f"qps{i}")
                nc.tensor.matmul(
                    ps[:, 0:512], lhsT=wq_s[:], rhs=xe[:, i * 512 : (i + 1) * 512],
                    start=True, stop=True,
                )
                nc.vector.tensor_copy(Qsb[:, i * 512 : (i + 1) * 512], ps[:, 0:512])

            def kproj(g, pool, copy_eng):
                off, nn = (HW, MEM) if g == 8 else (g * 512, 512)
                ps = pool.tile(
                    [128, 512] if pool is avp else [128, 1024],
                    F32, tag="avp" if pool is avp else "sim", name=f"kps{g}",
                )
                nc.tensor.matmul(
                    ps[:, 0:nn], lhsT=wk_s[:], rhs=xe[:, off : off + nn],
                    start=True, stop=True,
                )
                copy_eng(Ksb[:, off : off + nn], ps[:, 0:nn])

            def kproj2(g):
                # chunks g and g+1 share one sim tile; one 1024-wide copy
                off = g * 512
                ps = simp.tile([128, 1024], F32, tag="sim", name=f"kps2_{g}")
                for u in range(2):
                    nc.tensor.matmul(
                        ps[:, u * 512 : (u + 1) * 512], lhsT=wk_s[:],
                        rhs=xe[:, off + u * 512 : off + (u + 1) * 512],
                        start=True, stop=True,
                    )
                nc.vector.tensor_copy(Ksb[:, off : off + 1024], ps[:])

            # only what the first chunks need runs up front; the rest of the
            # projections ride inside the main loop's slack
            qproj(0)
            kproj(0, avp, nc.scalar.copy)
            kproj(1, avp, nc.scalar.copy)
            nc.vector.memset(VT[:, :, :, 32:33], 1.0)

            def vproj_group(g):
                # vT for token chunks 8g..8g+7 (g=4: the 16 memory tokens)
                if g == 4:
                    ps = simp.tile([128, 1024], F32, tag="sim", name="vps8")
                    nc.tensor.matmul(
                        ps[0:MEM, 0:128], lhsT=xe[:, HW:NT], rhs=wv_s[:],
                        start=True, stop=True,
                    )
                    nc.vector.tensor_copy(
                        VT[0:MEM, 32, :, 0:32],
                        ps[0:MEM, 0:128].rearrange("p (h d) -> p h d", h=HEADS, d=32),
                    )
                    return
                ps = simp.tile([128, 1024], F32, tag="sim", name=f"vps{g}")
                for i in range(8):
                    off = (8 * g + i) * 128
                    nc.tensor.matmul(
                        ps[:, i * 128 : (i + 1) * 128],
                        lhsT=xe[:, off : off + 128], rhs=wv_s[:],
                        start=True, stop=True,
                    )
                src = ps[:].rearrange("p (c h d) -> p c h d", c=8, h=HEADS, d=32)
                nc.vector.tensor_copy(VT[:, 8 * g : 8 * g + 8, :, 0:32], src)

            # ---- main attention loop -------------------------------------
            # Per k-chunk, head pair (2t, 2t+1) shares one (128,1024) score
            # tile; the two QK matmuls sit on distinct 32-row PE strips (they
            # share one XBUS stream of Qsb) and the two AV matmuls sit on
            # distinct col strips via tile_position (0,0)/(0,64), so each
            # pair runs concurrently on the array.
            def outproj(qb):
                qlo, qhi = qb * 512, (qb + 1) * 512
                op = simp.tile([128, 1024], F32, tag="sim", name=f"op{qb}")
                nc.tensor.matmul(
                    op[:, 0:512], lhsT=wo_s[:], rhs=att[:, qlo:qhi],
                    start=True, stop=True,
                )
                nc.vector.tensor_scalar_add(osb[:, qlo:qhi], op[:, 0:512], bo_s[:])
                nc.sync.dma_start(out=outd[:, qlo:qhi], in_=osb[:, qlo:qhi])

            # AV matmuls are emitted AV_LAG chunk-halves behind their exp so
            # the in-order PE stream never blocks on an accumulator slot
            # while the previous block's epilogue drains.
            AV_LAG = 10

            for qb in range(4):
                qlo, qhi = qb * 512, (qb + 1) * 512
                av = [
                    avp.tile([128, 512], F32, tag="avp", name=f"av{qb}_{t}")
                    for t in range(2)
                ]
                pending = []

                def emit_av(item):
                    t, j, nn, pt = item
                    for s in range(2):
                        h = 2 * t + s
                        nc.tensor.matmul(
                            av[t][64 * s : 64 * s + VTW, :],
                            lhsT=VT[0:nn, j, h, :],
                            rhs=pt[0:nn, s * 512 : (s + 1) * 512],
                            start=(j == 0), stop=(j == 32),
                            tile_position=(0, 64 * s),
                            skip_group_check=True,
                        )

                for j, (off, nn) in enumerate(KCHUNKS):
                    if qb == 0 and 2 <= j < 12 and j % 2 == 0:
                        vproj_group(j // 2 - 1)
                    if qb == 0 and j in (1, 9, 17):
                        kproj2(2 + (j // 8) * 2)
                    if qb == 0 and j == 25:
                        kproj(8, simp, nc.vector.tensor_copy)
                    if j == 16 and qb < 3:
                        qproj(qb + 1)
                    if j == 8 and qb > 0:
                        outproj(qb - 1)  # previous block's projection; PE has
                        # slack here and its deps are long satisfied
                    for t in range(2):
                        sim = simp.tile([128, 1024], F32, tag="sim", name=f"s{qb}_{j}_{t}")
                        for s in range(2):
                            h = 2 * t + s
                            nc.tensor.matmul(
                                sim[0:nn, s * 512 : (s + 1) * 512],
                                lhsT=Ksb[32 * h : 32 * (h + 1), off : off + nn],
                                rhs=Qsb[32 * h : 32 * (h + 1), qlo:qhi],
                                start=True, stop=True,
                                tile_position=(32 * h, 0),
                            )
                        pt = ptp.tile([128, 1024], B16, tag="pt", name=f"p{qb}_{j}_{t}")
                        nc.scalar.activation(pt[0:nn, :], sim[0:nn, :], EXP, scale=SCALE)
                        pending.append((t, j, nn, pt))
                        if len(pending) > AV_LAG:
                            emit_av(pending.pop(0))
                # normalize: att[32h+d, q] = av[d, q] / av[denom_row, q].
                # Flush the lagged AVs and run the epilogue per head pair so
                # t=0's DVE chain overlaps t=1's remaining AV matmuls.
                for t in range(2):
                    for item in [p for p in pending if p[0] == t]:
                        emit_av(item)
                    # same-base copies release the av psum slot fast
                    avn = epp.tile([128, 512], F32, tag="avn", name=f"avn{qb}_{t}")
                    nc.vector.tensor_copy(avn[0:33, :], av[t][0:33, :])
                    nc.vector.tensor_copy(avn[64:97, :], av[t][64:97, :])
                    for s in range(2):
                        h = 2 * t + s
                        lo = 64 * s
                        dnm = epp.tile([1, 512], F32, tag="dnm")
                        nc.vector.tensor_copy(dnm[:], avn[lo + 32 : lo + 33, :])
                        rcp = epp.tile([1, 512], F32, tag="rcp")
                        nc.vector.reciprocal_approx_fast(rcp[:], dnm[:])
                        # partition_broadcast writes zeros for out base != 0 on
                        # HW, so always broadcast from base 0 wide enough to
                        # cover this head's partition range
                        rrep = epp.tile([128, 512], F32, tag="rrep", name=f"rr{qb}_{t}{s}")
                        nc.gpsimd.partition_broadcast(rrep[0 : lo + 32, :], rcp[:])
                        nc.vector.tensor_mul(
                            att[32 * h : 32 * (h + 1), qlo:qhi],
                            avn[lo : lo + 32, :],
                            rrep[lo : lo + 32, :],
                        )
                    if qb == 3:
                        # tail: start the last output projection's first half
                        # as soon as heads 0/1 are normalized
                        op3 = op3_tile if t else simp.tile(
                            [128, 1024], F32, tag="sim", name="op3"
                        )
                        op3_tile = op3
                        nc.tensor.matmul(
                            op3[:, 0:512],
                            lhsT=wo_s[64 * t : 64 * (t + 1), :],
                            rhs=att[64 * t : 64 * (t + 1), 1536:2048],
                            start=(t == 0), stop=(t == 1),
                            tile_position=(64 * t, 0),
                        )
                pending.clear()

            nc.vector.tensor_scalar_add(osb[:, 1536:2048], op3_tile[:, 0:512], bo_s[:])
            nc.sync.dma_start(out=outd[:, 1536:2048], in_=osb[:, 1536:2048])

    nc.compile()
    return nc


def get_nc():
    global _NC_CACHE
    if _NC_CACHE is None:
        _NC_CACHE = _build_nc()
    return _NC_CACHE


def make_in_maps(x, memory, w_qkv, w_out, b_out):
    """Host-side shard prep. Core c = 2*b + half handles batch b, queries
    [half*2048, half*2048+2048). Tokens are rolled so each core's queries
    occupy columns [0, 2048) -- attention is permutation-invariant in keys,
    so K/V over the rolled token set give identical outputs."""
    x = np.ascontiguousarray(np.asarray(x, dtype=np.float32).reshape(B, C, HW))
    mem = np.ascontiguousarray(np.asarray(memory, dtype=np.float32).reshape(C, MEM))
    w_qkv = np.asarray(w_qkv, dtype=np.float32)
    w_out = np.asarray(w_out, dtype=np.float32)
    b_out = np.asarray(b_out, dtype=np.float32)

    wqT = np.ascontiguousarray(w_qkv[0:128].T)
    wkT = np.ascontiguousarray(w_qkv[128:256].T)
    wvT = np.ascontiguousarray(w_qkv[256:384].T)
    woT = np.ascontiguousarray(w_out.T)
    bo = np.ascontiguousarray(b_out.reshape(C, 1))

    in_maps = []
    for core in range(8):
        b, half = divmod(core, 2)
        xb = x[b] if half == 0 else np.ascontiguousarray(np.roll(x[b], -QN, axis=1))
        in_maps.append(
            {
                "x": xb,
                "mem": mem,
                "wqT": wqT,
                "wkT": wkT,
                "wvT": wvT,
                "woT": woT,
                "bout": bo,
            }
        )
    return in_maps


def assemble(results):
    """results: list of 8 dicts with per-core 'out' of shape (C, QN)."""
    out = np.empty((B, C, HW), dtype=np.float32)
    for core in range(8):
        b, half = divmod(core, 2)
        out[b, :, half * QN : (half + 1) * QN] = results[core]["out"]
    return out.reshape(B, C, H, W)


def kernel(x, memory, w_qkv, w_out, b_out, _trace=False):
    from concourse.bass_utils import run_bass_kernel_spmd

    nc = get_nc()
    in_maps = make_in_maps(x, memory, w_qkv, w_out, b_out)
    res = run_bass_kernel_spmd(nc, in_maps, core_ids=list(range(8)), trace=_trace)
    out = assemble(res.results)
    if _trace:
        return out, res
    return out


# revision 54
# speedup vs baseline: 1.0028x; 1.0028x over previous
"""Trainium2 Bass kernel for nn_Attention_39487929319718.

Module: x:(4,128,64,64) -> 1x1conv QKV (+16 memory tokens) -> 4-head
attention (d=32) over n=4112 tokens -> drop memory queries -> 1x1 conv out.

Sharding: 8 cores = 4 batches x 2 query-halves. Each core receives its
batch's full x (token-rolled so its own 2048 queries sit first), computes
K/V for all 4112 tokens, attention for its 2048 queries across all 4 heads,
and the full output projection for those tokens. No collectives.

Per-core layout highlights:
 - Scores are computed transposed (k-tokens on partitions) so softmax'd
   probs feed the AV matmul directly with no transposes.
 - exp() has no max-subtraction (|scores| < 3 for this data distribution),
   with the 1/sqrt(d) scale folded into the activation's free affine.
 - V^T carries an extra ones column so each AV accumulation also produces
   the softmax denominator row for free.
"""

import sys

sys.path.insert(0, "/opt/trn_rl_repo")

import numpy as np

B, C, H, W = 4, 128, 64, 64
HEADS, DH, MEM = 4, 32, 16
HW = H * W            # 4096
NT = HW + MEM         # 4112 tokens (keys/values)
QN = HW // 2          # 2048 queries per core
SCALE = float(DH) ** -0.5

# k-token chunks of 128 (last chunk = 16 memory tokens)
KCHUNKS = [(j * 128, 128) for j in range(32)] + [(4096, 16)]
VTW = 33  # per-head vT columns per chunk: 32 v dims + 1 ones
VCW = HEADS * VTW  # 132 columns per chunk in the vT tile

_NC_CACHE = None


def _build_nc():
    import concourse.bacc as bacc
    import concourse.mybir as mybir
    import concourse.tile as tile

    F32 = mybir.dt.float32
    B16 = mybir.dt.bfloat16
    EXP = mybir.ActivationFunctionType.Exp

    nc = bacc.Bacc("TRN2", target_bir_lowering=False, debug=False, num_devices=8)

    xd = nc.dram_tensor("x", [C, HW], F32, kind="ExternalInput").ap()
    memd = nc.dram_tensor("mem", [C, MEM], F32, kind="ExternalInput").ap()
    wq_d = nc.dram_tensor("wqT", [C, 128], F32, kind="ExternalInput").ap()
    wk_d = nc.dram_tensor("wkT", [C, 128], F32, kind="ExternalInput").ap()
    wv_d = nc.dram_tensor("wvT", [C, 128], F32, kind="ExternalInput").ap()
    wo_d = nc.dram_tensor("woT", [128, C], F32, kind="ExternalInput").ap()
    bo_d = nc.dram_tensor("bout", [C, 1], F32, kind="ExternalInput").ap()
    outd = nc.dram_tensor("out", [C, QN], F32, kind="ExternalOutput").ap()

    with tile.TileContext(nc) as tc:
        with (
            tc.tile_pool(name="const", bufs=1) as constp,
            tc.tile_pool(name="big", bufs=1) as bigp,
            tc.tile_pool(name="pt", bufs=12) as ptp,
            tc.tile_pool(name="xin", bufs=3) as xinp,
            tc.tile_pool(name="ep", bufs=6) as epp,
            tc.tile_pool(name="simps", bufs=3, space="PSUM") as simp,
            tc.tile_pool(name="avps", bufs=2, space="PSUM") as avp,
        ):
            # ---- constants / weights (QKV weights cast to bf16) ----------
            # The first x chunk and the K/Q weights ride the sync DMA queue
            # (they gate the score stream); everything else takes the gpsimd
            # queue so the critical chain isn't serialized behind it.
            xe = bigp.tile([C, NT], B16, tag="xe")
            xf0 = xinp.tile([C, 512], F32, tag="xf", name="xf0")
            nc.sync.dma_start(out=xf0[:], in_=xd[:, 0:512])
            nc.vector.tensor_copy(xe[:, 0:512], xf0[:])

            wk_f = constp.tile([C, 128], F32, tag="wkf")
            nc.sync.dma_start(out=wk_f[:], in_=wk_d)
            wk_s = constp.tile([C, 128], B16, tag="wk")
            nc.vector.tensor_copy(wk_s[:], wk_f[:])
            wq_f = constp.tile([C, 128], F32, tag="wqf")
            nc.sync.dma_start(out=wq_f[:], in_=wq_d)
            wq_s = constp.tile([C, 128], B16, tag="wq")
            nc.vector.tensor_copy(wq_s[:], wq_f[:])
            wv_f = constp.tile([C, 128], F32, tag="wvf")
            nc.gpsimd.dma_start(out=wv_f[:], in_=wv_d)
            wv_s = constp.tile([C, 128], B16, tag="wv")
            nc.vector.tensor_copy(wv_s[:], wv_f[:])
            wo_f = constp.tile([128, C], F32, tag="wof")
            nc.gpsimd.dma_start(out=wo_f[:], in_=wo_d)
            wo_s = constp.tile([128, C], B16, tag="wo")
            nc.vector.tensor_copy(wo_s[:], wo_f[:])
            bo_s = constp.tile([C, 1], F32, tag="bo")
            nc.gpsimd.dma_start(out=bo_s[:], in_=bo_d)

            # ---- rest of x_ext, cast to bf16 chunk by chunk --------------
            for i in range(1, 8):
                xf = xinp.tile([C, 512], F32, tag="xf", name=f"xf{i}")
                eng = nc.sync if i % 2 == 0 else nc.gpsimd
                eng.dma_start(out=xf[:], in_=xd[:, i * 512 : (i + 1) * 512])
                nc.vector.tensor_copy(xe[:, i * 512 : (i + 1) * 512], xf[:])
            mf = xinp.tile([C, MEM], F32, tag="mf")
            nc.gpsimd.dma_start(out=mf[:], in_=memd)
            nc.vector.tensor_copy(xe[:, HW:NT], mf[:])

            # main-loop matmul operands in bf16 (accumulation stays fp32)
            Ksb = bigp.tile([C, NT], B16, tag="K")          # (4h x 32d, tok)
            Qsb = bigp.tile([C, QN], B16, tag="Q")          # (4h x 32d, q)
            VT = bigp.tile([C, 33, HEADS, VTW], B16, tag="VT")  # (tok%128, chunk, h, d+1)
            att = bigp.tile([C, QN], B16, tag="att")        # (4h x 32d, q) normalized
            osb = bigp.tile([C, QN], F32, tag="osb")

            # ---- projections ---------------------------------------------
            # Q: first 2048 columns (this core's queries);  K: all 4112 tokens
            # K/Q psum evacuation on ScalarE (idle during setup; DVE is busy
            # with the vT copies which gate the AV stream)
            # Projection psum comes from the sim pool; the avp pool is
            # reserved for AV accumulators so the AV stream starts at once.
            # K projections run through the avp pool (idle until the first
            # lagged AV fires) with psum evacuation on ScalarE; Q through the
            # sim pool on DVE. V-projection groups are emitted inside the
            # first query block's chunk loop so the score/exp stream starts
            # immediately and setup rides in PE slack.
            def qproj(i):
                ps = simp.tile([128, 1024], F32, tag="sim", name=f"qps{i}")
                nc.tensor.matmul(
                    ps[:, 0:512], lhsT=wq_s[:], rhs=xe[:, i * 512 : (i + 1) * 512],
                    start=True, stop=True,
                )
                nc.vector.tensor_copy(Qsb[:, i * 512 : (i + 1) * 512], ps[:, 0:512])

            def kproj(g, pool, copy_eng):
                off, nn = (HW, MEM) if g == 8 else (g * 512, 512)
                ps = pool.tile(
                    [128, 512] if pool is avp else [128, 1024],
                    F32, tag="avp" if pool is avp else "sim", name=f"kps{g}",
                )
                nc.tensor.matmul(
                    ps[:, 0:nn], lhsT=wk_s[:], rhs=xe[:, off : off + nn],
                    start=True, stop=True,
                )
                copy_eng(Ksb[:, off : off + nn], ps[:, 0:nn])

            def kproj2(g):
                # chunks g and g+1 share one sim tile; one 1024-wide copy
                off = g * 512
                ps = simp.tile([128, 1024], F32, tag="sim", name=f"kps2_{g}")
                for u in range(2):
                    nc.tensor.matmul(
                        ps[:, u * 512 : (u + 1) * 512], lhsT=wk_s[:],
                        rhs=xe[:, off + u * 512 : off + (u + 1) * 512],
                        start=True, stop=True,
                    )
                nc.vector.tensor_copy(Ksb[:, off : off + 1024], ps[:])

            # only what the first chunks need runs up front; the rest of the
            # projections ride inside the main loop's slack
            qproj(0)
            kproj(0, avp, nc.scalar.copy)
            kproj(1, avp, nc.scalar.copy)
            nc.vector.memset(VT[:, :, :, 32:33], 1.0)

            def vproj_group(g):
                # vT for token chunks 8g..8g+7 (g=4: the 16 memory tokens)
                if g == 4:
                    ps = simp.tile([128, 1024], F32, tag="sim", name="vps8")
                    nc.tensor.matmul(
                        ps[0:MEM, 0:128], lhsT=xe[:, HW:NT], rhs=wv_s[:],
                        start=True, stop=True,
                    )
                    nc.vector.tensor_copy(
                        VT[0:MEM, 32, :, 0:32],
                        ps[0:MEM, 0:128].rearrange("p (h d) -> p h d", h=HEADS, d=32),
                    )
                    return
                ps = simp.tile([128, 1024], F32, tag="sim", name=f"vps{g}")
                for i in range(8):
                    off = (8 * g + i) * 128
                    nc.tensor.matmul(
                        ps[:, i * 128 : (i + 1) * 128],
                        lhsT=xe[:, off : off + 128], rhs=wv_s[:],
                        start=True, stop=True,
                    )
                src = ps[:].rearrange("p (c h d) -> p c h d", c=8, h=HEADS, d=32)
                nc.vector.tensor_copy(VT[:, 8 * g : 8 * g + 8, :, 0:32], src)

            # ---- main attention loop -------------------------------------
            # Per k-chunk, head pair (2t, 2t+1) shares one (128,1024) score
            # tile; the two QK matmuls sit on distinct 32-row PE strips (they
            # share one XBUS stream of Qsb) and the two AV matmuls sit on
            # distinct col strips via tile_position (0,0)/(0,64), so each
            # pair runs concurrently on the array.
            def outproj(qb):
                qlo, qhi = qb * 512, (qb + 1) * 512
                op = simp.tile([128, 1024], F32, tag="sim", name=f"op{qb}")
                nc.tensor.matmul(
                    op[:, 0:512], lhsT=wo_s[:], rhs=att[:, qlo:qhi],
                    start=True, stop=True,
                )
                nc.vector.tensor_scalar_add(osb[:, qlo:qhi], op[:, 0:512], bo_s[:])
                nc.sync.dma_start(out=outd[:, qlo:qhi], in_=osb[:, qlo:qhi])

            # AV matmuls are emitted AV_LAG chunk-halves behind their exp so
            # the in-order PE stream never blocks on an accumulator slot
            # while the previous block's epilogue drains.
            AV_LAG = 10

            for qb in range(4):
                qlo, qhi = qb * 512, (qb + 1) * 512
                av = [
                    avp.tile([128, 512], F32, tag="avp", name=f"av{qb}_{t}")
                    for t in range(2)
                ]
                pending = []

                def emit_av(item):
                    t, j, nn, pt = item
                    for s in range(2):
                        h = 2 * t + s
                        nc.tensor.matmul(
                            av[t][64 * s : 64 * s + VTW, :],
                            lhsT=VT[0:nn, j, h, :],
                            rhs=pt[0:nn, s * 512 : (s + 1) * 512],
                            start=(j == 0), stop=(j == 32),
                            tile_position=(0, 64 * s),
                            skip_group_check=True,
                        )

                for j, (off, nn) in enumerate(KCHUNKS):
                    if qb == 0 and 2 <= j < 12 and j % 2 == 0:
                        vproj_group(j // 2 - 1)
                    if qb == 0 and j in (1, 9, 17):
                        kproj2(2 + (j // 8) * 2)
                    if qb == 0 and j == 25:
                        kproj(8, simp, nc.vector.tensor_copy)
                    if j == 16 and qb < 3:
                        qproj(qb + 1)
                    if j == 8 and qb > 0:
                        outproj(qb - 1)  # previous block's projection; PE has
                        # slack here and its deps are long satisfied
                    for t in range(2):
                        sim = simp.tile([128, 1024], F32, tag="sim", name=f"s{qb}_{j}_{t}")
                        for s in range(2):
                            h = 2 * t + s
                            nc.tensor.matmul(
                                sim[0:nn, s * 512 : (s + 1) * 512],
                                lhsT=Ksb[32 * h : 32 * (h + 1), off : off + nn],
                                rhs=Qsb[32 * h : 32 * (h + 1), qlo:qhi],
                                start=True, stop=True,
                                tile_position=(32 * h, 0),
                            )
                        pt = ptp.tile([128, 1024], B16, tag="pt", name=f"p{qb}_{j}_{t}")
                        nc.scalar.activation(pt[0:nn, :], sim[0:nn, :], EXP, scale=SCALE)
                        pending.append((t, j, nn, pt))
                        if len(pending) > AV_LAG:
                            emit_av(pending.pop(0))
                # normalize: att[32h+d, q] = av[d, q] / av[denom_row, q].
                # Flush the lagged AVs and run the epilogue per head pair so
                # t=0's DVE chain overlaps t=1's remaining AV matmuls.
                for t in range(2):
                    for item in [p for p in pending if p[0] == t]:
                        emit_av(item)
                    # same-base copies release the av psum slot fast
                    avn = epp.tile([128, 512], F32, tag="avn", name=f"avn{qb}_{t}")
                    nc.vector.tensor_copy(avn[0:33, :], av[t][0:33, :])
                    nc.vector.tensor_copy(avn[64:97, :], av[t][64:97, :])
                    for s in range(2):
                        h = 2 * t + s
                        lo = 64 * s
                        dnm = epp.tile([1, 512], F32, tag="dnm")
                        nc.vector.tensor_copy(dnm[:], avn[lo + 32 : lo + 33, :])
                        rcp = epp.tile([1, 512], F32, tag="rcp")
                        nc.vector.reciprocal_approx_fast(rcp[:], dnm[:])
                        # partition_broadcast writes zeros for out base != 0 on
                        # HW, so always broadcast from base 0 wide enough to
                        # cover this head's partition range
                        rrep = epp.tile([128, 512], F32, tag="rrep", name=f"rr{qb}_{t}{s}")
                        nc.gpsimd.partition_broadcast(rrep[0 : lo + 32, :], rcp[:])
                        nc.vector.tensor_mul(
                            att[32 * h : 32 * (h + 1), qlo:qhi],
                            avn[lo : lo + 32, :],
                            rrep[lo : lo + 32, :],
                        )
                pending.clear()

            outproj(3)

    nc.compile()
    return nc


def get_nc():
    global _NC_CACHE
    if _NC_CACHE is None:
        _NC_CACHE = _build_nc()
    return _NC_CACHE


def make_in_maps(x, memory, w_qkv, w_out, b_out):
    """Host-side shard prep. Core c = 2*b + half handles batch b, queries
    [half*2048, half*2048+2048). Tokens are rolled so each core's queries
    occupy columns [0, 2048) -- attention is permutation-invariant in keys,
    so K/V over the rolled token set give identical outputs."""
    x = np.ascontiguousarray(np.asarray(x, dtype=np.float32).reshape(B, C, HW))
    mem = np.ascontiguousarray(np.asarray(memory, dtype=np.float32).reshape(C, MEM))
    w_qkv = np.asarray(w_qkv, dtype=np.float32)
    w_out = np.asarray(w_out, dtype=np.float32)
    b_out = np.asarray(b_out, dtype=np.float32)

    wqT = np.ascontiguousarray(w_qkv[0:128].T)
    wkT = np.ascontiguousarray(w_qkv[128:256].T)
    wvT = np.ascontiguousarray(w_qkv[256:384].T)
    woT = np.ascontiguousarray(w_out.T)
    bo = np.ascontiguousarray(b_out.reshape(C, 1))

    in_maps = []
    for core in range(8):
        b, half = divmod(core, 2)
        xb = x[b] if half == 0 else np.ascontiguousarray(np.roll(x[b], -QN, axis=1))
        in_maps.append(
            {
                "x": xb,
                "mem": mem,
                "wqT": wqT,
                "wkT": wkT,
                "wvT": wvT,
                "woT": woT,
                "bout": bo,
            }
        )
    return in_maps


def assemble(results):
    """results: list of 8 dicts with per-core 'out' of shape (C, QN)."""
    out = np.empty((B, C, HW), dtype=np.float32)
    for core in range(8):
        b, half = divmod(core, 2)
        out[b, :, half * QN : (half + 1) * QN] = results[core]["out"]
    return out.reshape(B, C, H, W)


def kernel(x, memory, w_qkv, w_out, b_out, _trace=False):
    from concourse.bass_utils import run_bass_kernel_spmd

    nc = get_nc()
    in_maps = make_in_maps(x, memory, w_qkv, w_out, b_out)
    res = run_bass_kernel_spmd(nc, in_maps, core_ids=list(range(8)), trace=_trace)
    out = assemble(res.results)
    if _trace:
        return out, res
    return out


# revision 55
# speedup vs baseline: 1.0032x; 1.0004x over previous
"""Trainium2 Bass kernel for nn_Attention_39487929319718.

Module: x:(4,128,64,64) -> 1x1conv QKV (+16 memory tokens) -> 4-head
attention (d=32) over n=4112 tokens -> drop memory queries -> 1x1 conv out.

Sharding: 8 cores = 4 batches x 2 query-halves. Each core receives its
batch's full x (token-rolled so its own 2048 queries sit first), computes
K/V for all 4112 tokens, attention for its 2048 queries across all 4 heads,
and the full output projection for those tokens. No collectives.

Per-core layout highlights:
 - Scores are computed transposed (k-tokens on partitions) so softmax'd
   probs feed the AV matmul directly with no transposes.
 - exp() has no max-subtraction (|scores| < 3 for this data distribution),
   with the 1/sqrt(d) scale folded into the activation's free affine.
 - V^T carries an extra ones column so each AV accumulation also produces
   the softmax denominator row for free.
"""

import sys

sys.path.insert(0, "/opt/trn_rl_repo")

import numpy as np

B, C, H, W = 4, 128, 64, 64
HEADS, DH, MEM = 4, 32, 16
HW = H * W            # 4096
NT = HW + MEM         # 4112 tokens (keys/values)
QN = HW // 2          # 2048 queries per core
SCALE = float(DH) ** -0.5

# k-token chunks of 128 (last chunk = 16 memory tokens)
KCHUNKS = [(j * 128, 128) for j in range(32)] + [(4096, 16)]
VTW = 33  # per-head vT columns per chunk: 32 v dims + 1 ones
VCW = HEADS * VTW  # 132 columns per chunk in the vT tile

_NC_CACHE = None


def _build_nc():
    import concourse.bacc as bacc
    import concourse.mybir as mybir
    import concourse.tile as tile

    F32 = mybir.dt.float32
    B16 = mybir.dt.bfloat16
    EXP = mybir.ActivationFunctionType.Exp

    nc = bacc.Bacc("TRN2", target_bir_lowering=False, debug=False, num_devices=8)

    xd = nc.dram_tensor("x", [C, HW], F32, kind="ExternalInput").ap()
    memd = nc.dram_tensor("mem", [C, MEM], F32, kind="ExternalInput").ap()
    wq_d = nc.dram_tensor("wqT", [C, 128], F32, kind="ExternalInput").ap()
    wk_d = nc.dram_tensor("wkT", [C, 128], F32, kind="ExternalInput").ap()
    wv_d = nc.dram_tensor("wvT", [C, 128], F32, kind="ExternalInput").ap()
    wo_d = nc.dram_tensor("woT", [128, C], F32, kind="ExternalInput").ap()
    bo_d = nc.dram_tensor("bout", [C, 1], F32, kind="ExternalInput").ap()
    outd = nc.dram_tensor("out", [C, QN], F32, kind="ExternalOutput").ap()

    with tile.TileContext(nc) as tc:
        with (
            tc.tile_pool(name="const", bufs=1) as constp,
            tc.tile_pool(name="big", bufs=1) as bigp,
            tc.tile_pool(name="pt", bufs=12) as ptp,
            tc.tile_pool(name="xin", bufs=3) as xinp,
            tc.tile_pool(name="ep", bufs=6) as epp,
            tc.tile_pool(name="simps", bufs=3, space="PSUM") as simp,
            tc.tile_pool(name="avps", bufs=2, space="PSUM") as avp,
        ):
            # ---- constants / weights (QKV weights cast to bf16) ----------
            # The first x chunk and the K/Q weights ride the sync DMA queue
            # (they gate the score stream); everything else takes the gpsimd
            # queue so the critical chain isn't serialized behind it.
            xe = bigp.tile([C, NT], B16, tag="xe")
            xf0 = xinp.tile([C, 512], F32, tag="xf", name="xf0")
            nc.sync.dma_start(out=xf0[:], in_=xd[:, 0:512])
            nc.vector.tensor_copy(xe[:, 0:512], xf0[:])

            wk_f = constp.tile([C, 128], F32, tag="wkf")
            nc.sync.dma_start(out=wk_f[:], in_=wk_d)
            wk_s = constp.tile([C, 128], B16, tag="wk")
            nc.vector.tensor_copy(wk_s[:], wk_f[:])
            wq_f = constp.tile([C, 128], F32, tag="wqf")
            nc.sync.dma_start(out=wq_f[:], in_=wq_d)
            wq_s = constp.tile([C, 128], B16, tag="wq")
            nc.vector.tensor_copy(wq_s[:], wq_f[:])
            wv_f = constp.tile([C, 128], F32, tag="wvf")
            nc.gpsimd.dma_start(out=wv_f[:], in_=wv_d)
            wv_s = constp.tile([C, 128], B16, tag="wv")
            nc.vector.tensor_copy(wv_s[:], wv_f[:])
            wo_f = constp.tile([128, C], F32, tag="wof")
            nc.gpsimd.dma_start(out=wo_f[:], in_=wo_d)
            wo_s = constp.tile([128, C], B16, tag="wo")
            nc.vector.tensor_copy(wo_s[:], wo_f[:])
            bo_s = constp.tile([C, 1], F32, tag="bo")
            nc.gpsimd.dma_start(out=bo_s[:], in_=bo_d)

            # ---- rest of x_ext, cast to bf16 chunk by chunk --------------
            for i in range(1, 8):
                xf = xinp.tile([C, 512], F32, tag="xf", name=f"xf{i}")
                eng = nc.sync if i % 2 == 0 else nc.gpsimd
                eng.dma_start(out=xf[:], in_=xd[:, i * 512 : (i + 1) * 512])
                nc.vector.tensor_copy(xe[:, i * 512 : (i + 1) * 512], xf[:])
            mf = xinp.tile([C, MEM], F32, tag="mf")
            nc.gpsimd.dma_start(out=mf[:], in_=memd)
            nc.vector.tensor_copy(xe[:, HW:NT], mf[:])

            # main-loop matmul operands in bf16 (accumulation stays fp32)
            Ksb = bigp.tile([C, NT], B16, tag="K")          # (4h x 32d, tok)
            Qsb = bigp.tile([C, QN], B16, tag="Q")          # (4h x 32d, q)
            VT = bigp.tile([C, 33, HEADS, VTW], B16, tag="VT")  # (tok%128, chunk, h, d+1)
            att = bigp.tile([C, QN], B16, tag="att")        # (4h x 32d, q) normalized
            osb = bigp.tile([C, QN], F32, tag="osb")

            # ---- projections ---------------------------------------------
            # Q: first 2048 columns (this core's queries);  K: all 4112 tokens
            # K/Q psum evacuation on ScalarE (idle during setup; DVE is busy
            # with the vT copies which gate the AV stream)
            # Projection psum comes from the sim pool; the avp pool is
            # reserved for AV accumulators so the AV stream starts at once.
            # K projections run through the avp pool (idle until the first
            # lagged AV fires) with psum evacuation on ScalarE; Q through the
            # sim pool on DVE. V-projection groups are emitted inside the
            # first query block's chunk loop so the score/exp stream starts
            # immediately and setup rides in PE slack.
            def qproj(i):
                ps = simp.tile([128, 1024], F32, tag="sim", name=f"qps{i}")
                nc.tensor.matmul(
                    ps[:, 0:512], lhsT=wq_s[:], rhs=xe[:, i * 512 : (i + 1) * 512],
                    start=True, stop=True,
                )
                nc.vector.tensor_copy(Qsb[:, i * 512 : (i + 1) * 512], ps[:, 0:512])

            def kproj(g, pool, copy_eng):
                off, nn = (HW, MEM) if g == 8 else (g * 512, 512)
                ps = pool.tile(
                    [128, 512] if pool is avp else [128, 1024],
                    F32, tag="avp" if pool is avp else "sim", name=f"kps{g}",
                )
                nc.tensor.matmul(
                    ps[:, 0:nn], lhsT=wk_s[:], rhs=xe[:, off : off + nn],
                    start=True, stop=True,
                )
                copy_eng(Ksb[:, off : off + nn], ps[:, 0:nn])

            def kproj2(g):
                # chunks g and g+1 share one sim tile; one 1024-wide copy
                off = g * 512
                ps = simp.tile([128, 1024], F32, tag="sim", name=f"kps2_{g}")
                for u in range(2):
                    nc.tensor.matmul(
                        ps[:, u * 512 : (u + 1) * 512], lhsT=wk_s[:],
                        rhs=xe[:, off + u * 512 : off + (u + 1) * 512],
                        start=True, stop=True,
                    )
                nc.vector.tensor_copy(Ksb[:, off : off + 1024], ps[:])

            # only what the first chunks need runs up front; the rest of the
            # projections ride inside the main loop's slack
            qproj(0)
            kproj(0, avp, nc.scalar.copy)
            kproj(1, avp, nc.scalar.copy)
            nc.vector.memset(VT[:, :, :, 32:33], 1.0)

            def vproj_group(g):
                # vT for token chunks 8g..8g+7 (g=4: the 16 memory tokens)
                if g == 4:
                    ps = simp.tile([128, 1024], F32, tag="sim", name="vps8")
                    nc.tensor.matmul(
                        ps[0:MEM, 0:128], lhsT=xe[:, HW:NT], rhs=wv_s[:],
                        start=True, stop=True,
                    )
                    nc.vector.tensor_copy(
                        VT[0:MEM, 32, :, 0:32],
                        ps[0:MEM, 0:128].rearrange("p (h d) -> p h d", h=HEADS, d=32),
                    )
                    return
                ps = simp.tile([128, 1024], F32, tag="sim", name=f"vps{g}")
                for i in range(8):
                    off = (8 * g + i) * 128
                    nc.tensor.matmul(
                        ps[:, i * 128 : (i + 1) * 128],
                        lhsT=xe[:, off : off + 128], rhs=wv_s[:],
                        start=True, stop=True,
                    )
                src = ps[:].rearrange("p (c h d) -> p c h d", c=8, h=HEADS, d=32)
                nc.vector.tensor_copy(VT[:, 8 * g : 8 * g + 8, :, 0:32], src)

            # ---- main attention loop -------------------------------------
            # Per k-chunk, head pair (2t, 2t+1) shares one (128,1024) score
            # tile; the two QK matmuls sit on distinct 32-row PE strips (they
            # share one XBUS stream of Qsb) and the two AV matmuls sit on
            # distinct col strips via tile_position (0,0)/(0,64), so each
            # pair runs concurrently on the array.
            def outproj(qb):
                qlo, qhi = qb * 512, (qb + 1) * 512
                op = simp.tile([128, 1024], F32, tag="sim", name=f"op{qb}")
                nc.tensor.matmul(
                    op[:, 0:512], lhsT=wo_s[:], rhs=att[:, qlo:qhi],
                    start=True, stop=True,
                )
                nc.vector.tensor_scalar_add(osb[:, qlo:qhi], op[:, 0:512], bo_s[:])
                nc.sync.dma_start(out=outd[:, qlo:qhi], in_=osb[:, qlo:qhi])

            # AV matmuls are emitted AV_LAG chunk-halves behind their exp so
            # the in-order PE stream never blocks on an accumulator slot
            # while the previous block's epilogue drains.
            AV_LAG = 6

            for qb in range(4):
                qlo, qhi = qb * 512, (qb + 1) * 512
                av = [
                    avp.tile([128, 512], F32, tag="avp", name=f"av{qb}_{t}")
                    for t in range(2)
                ]
                pending = []

                def emit_av(item):
                    t, j, nn, pt = item
                    for s in range(2):
                        h = 2 * t + s
                        nc.tensor.matmul(
                            av[t][64 * s : 64 * s + VTW, :],
                            lhsT=VT[0:nn, j, h, :],
                            rhs=pt[0:nn, s * 512 : (s + 1) * 512],
                            start=(j == 0), stop=(j == 32),
                            tile_position=(0, 64 * s),
                            skip_group_check=True,
                        )

                for j, (off, nn) in enumerate(KCHUNKS):
                    if qb == 0 and 2 <= j < 12 and j % 2 == 0:
                        vproj_group(j // 2 - 1)
                    if qb == 0 and j in (1, 9, 17):
                        kproj2(2 + (j // 8) * 2)
                    if qb == 0 and j == 25:
                        kproj(8, simp, nc.vector.tensor_copy)
                    if j == 16 and qb < 3:
                        qproj(qb + 1)
                    if j == 8 and qb > 0:
                        outproj(qb - 1)  # previous block's projection; PE has
                        # slack here and its deps are long satisfied
                    for t in range(2):
                        sim = simp.tile([128, 1024], F32, tag="sim", name=f"s{qb}_{j}_{t}")
                        for s in range(2):
                            h = 2 * t + s
                            nc.tensor.matmul(
                                sim[0:nn, s * 512 : (s + 1) * 512],
                                lhsT=Ksb[32 * h : 32 * (h + 1), off : off + nn],
                                rhs=Qsb[32 * h : 32 * (h + 1), qlo:qhi],
                                start=True, stop=True,
                                tile_position=(32 * h, 0),
                            )
                        pt = ptp.tile([128, 1024], B16, tag="pt", name=f"p{qb}_{j}_{t}")
                        nc.scalar.activation(pt[0:nn, :], sim[0:nn, :], EXP, scale=SCALE)
                        pending.append((t, j, nn, pt))
                        if len(pending) > AV_LAG:
                            emit_av(pending.pop(0))
                # normalize: att[32h+d, q] = av[d, q] / av[denom_row, q].
                # Flush the lagged AVs and run the epilogue per head pair so
                # t=0's DVE chain overlaps t=1's remaining AV matmuls.
                for t in range(2):
                    for item in [p for p in pending if p[0] == t]:
                        emit_av(item)
                    # same-base copies release the av psum slot fast
                    avn = epp.tile([128, 512], F32, tag="avn", name=f"avn{qb}_{t}")
                    nc.vector.tensor_copy(avn[0:33, :], av[t][0:33, :])
                    nc.vector.tensor_copy(avn[64:97, :], av[t][64:97, :])
                    for s in range(2):
                        h = 2 * t + s
                        lo = 64 * s
                        dnm = epp.tile([1, 512], F32, tag="dnm")
                        nc.vector.tensor_copy(dnm[:], avn[lo + 32 : lo + 33, :])
                        rcp = epp.tile([1, 512], F32, tag="rcp")
                        nc.vector.reciprocal_approx_fast(rcp[:], dnm[:])
                        # partition_broadcast writes zeros for out base != 0 on
                        # HW, so always broadcast from base 0 wide enough to
                        # cover this head's partition range
                        rrep = epp.tile([128, 512], F32, tag="rrep", name=f"rr{qb}_{t}{s}")
                        nc.gpsimd.partition_broadcast(rrep[0 : lo + 32, :], rcp[:])
                        nc.vector.tensor_mul(
                            att[32 * h : 32 * (h + 1), qlo:qhi],
                            avn[lo : lo + 32, :],
                            rrep[lo : lo + 32, :],
                        )
                pending.clear()

            outproj(3)

    nc.compile()
    return nc


def get_nc():
    global _NC_CACHE
    if _NC_CACHE is None:
        _NC_CACHE = _build_nc()
    return _NC_CACHE


def make_in_maps(x, memory, w_qkv, w_out, b_out):
    """Host-side shard prep. Core c = 2*b + half handles batch b, queries
    [half*2048, half*2048+2048). Tokens are rolled so each core's queries
    occupy columns [0, 2048) -- attention is permutation-invariant in keys,
    so K/V over the rolled token set give identical outputs."""
    x = np.ascontiguousarray(np.asarray(x, dtype=np.float32).reshape(B, C, HW))
    mem = np.ascontiguousarray(np.asarray(memory, dtype=np.float32).reshape(C, MEM))
    w_qkv = np.asarray(w_qkv, dtype=np.float32)
    w_out = np.asarray(w_out, dtype=np.float32)
    b_out = np.asarray(b_out, dtype=np.float32)

    wqT = np.ascontiguousarray(w_qkv[0:128].T)
    wkT = np.ascontiguousarray(w_qkv[128:256].T)
    wvT = np.ascontiguousarray(w_qkv[256:384].T)
    woT = np.ascontiguousarray(w_out.T)
    bo = np.ascontiguousarray(b_out.reshape(C, 1))

    in_maps = []
    for core in range(8):
        b, half = divmod(core, 2)
        xb = x[b] if half == 0 else np.ascontiguousarray(np.roll(x[b], -QN, axis=1))
        in_maps.append(
            {
                "x": xb,
                "mem": mem,
                "wqT": wqT,
                "wkT": wkT,
                "wvT": wvT,
                "woT": woT,
                "bout": bo,
            }
        )
    return in_maps


def assemble(results):
    """results: list of 8 dicts with per-core 'out' of shape (C, QN)."""
    out = np.empty((B, C, HW), dtype=np.float32)
    for core in range(8):
        b, half = divmod(core, 2)
        out[b, :, half * QN : (half + 1) * QN] = results[core]["out"]
    return out.reshape(B, C, H, W)


def kernel(x, memory, w_qkv, w_out, b_out, _trace=False):
    from concourse.bass_utils import run_bass_kernel_spmd

    nc = get_nc()
    in_maps = make_in_maps(x, memory, w_qkv, w_out, b_out)
    res = run_bass_kernel_spmd(nc, in_maps, core_ids=list(range(8)), trace=_trace)
    out = assemble(res.results)
    if _trace:
        return out, res
    return out
